# revision 33
# baseline (speedup 1.0000x reference)
"""Multi-head attention (B=2, N=2048, D=2048, 16 heads) on 8 NeuronCores.

Sharding: tensor-parallel over heads (2 heads/core) for QKV projections and
attention; one AllToAll per (head, batch) re-shards the attention context
from head-split to row-split; the output projection is row-parallel
(512 rows/core) with the full Wo staged in SBUF on every core.

Performance notes (traced on TRN2; ~490us vs 654us f32r baseline):
  - all operands bf16 (halves DMA/SBUF/collective traffic; PE streams
    ~1 col/cycle either way; rel-err budget 2e-2 dwarfs bf16 rounding)
  - attention kc loop runs 4 matmuls (2 scores + 2 PV); the softmax
    denominators accumulate as pt-tile running sums on the DVE (bf16 2x
    mode) with a single ones-matmul per query chunk into a spare st slot,
    keeping the PE/ACT cadence balanced at ~1.4us per kc
  - 1/colsum via reciprocal_approx_fast (~0.7us vs 4us iterative divide)
  - the normalize+A2A-write stage is deferred one qc pair so the
    reciprocal DRAM-bounce broadcast hides under the next 16us kc loop;
    the bounce rides the sync queue so collectives never block it
  - batch-0 re-shards at 50% of attention; its ctxl loads issue on gpsimd
    a quarter after each A2A so no queue ever head-of-line blocks
  - phase 3 is stationary-outer (one ctxl LDWEIGHTS feeds 4 jc matmuls,
    8 PSUM accumulators); out is written bf16 and upcast on host
  - N=512 warmup matmuls span the startup DMA window so the PE HAM clock
    gate is open when real work lands; weight chunks stream just-in-time
    on the scalar queue while x tiles alternate sync/scalar queues

Layout strategy (everything contracts on the SBUF partition axis):
  - host feeds xT = x.T so projections need no on-device transposes
  - Q, K are produced transposed ([head_dim, rows]); V in natural layout by
    using the x-tile as the stationary operand
  - scores are computed transposed: S.T[k_row, q_row] = (K.T)^T . Q.T chunks
  - softmax skips the max-subtraction (scores ~ N(0,1); exp is safe)
  - ctx.T = v^T . P.T accumulates over k_row chunks -> ctx arrives transposed,
    which is exactly what the output projection needs
  - v-bias and o-bias commute out of the kernel: attention rows sum to 1, so
    out = attn@(v0+bv)@Wo.T + bo = device_out + (Wo@bv + bo); host adds it.
"""

import numpy as np
import ml_dtypes

import concourse.bacc as bacc
import concourse.mybir as mybir
import concourse.tile as tile
from concourse.bass_utils import run_bass_kernel_spmd

P = 128          # partitions
B = 2            # batch
SEQ = 2048       # sequence length
D = 2048         # hidden
H = 16           # heads
HD = D // H      # head dim = 128
W = 8            # cores
HPC = H // W     # heads per core = 2
DPC = HPC * HD   # features per core = 256
RPC = B * SEQ // W   # rows per core after re-shard = 512
FC = D // P      # feature chunks = 16
RT = B * SEQ     # total rows = 4096
KRC = SEQ // P   # key-row chunks per batch = 16
QRC = SEQ // 512  # query chunks of 512 per batch = 4

f32 = mybir.dt.float32
bf16 = mybir.dt.bfloat16
BF = ml_dtypes.bfloat16

INV_SQRT_HD = 1.0 / float(np.sqrt(HD))
Act = mybir.ActivationFunctionType

_CACHED_NC = None


def build_nc():
    nc = bacc.Bacc("TRN2", target_bir_lowering=False, debug=False)

    xT = nc.dram_tensor("xT", [D, RT], bf16, kind="ExternalInput")
    # weight chunks packed host-side: [p, fc, dpc]
    wq = nc.dram_tensor("wq", [P, FC, DPC], bf16, kind="ExternalInput")
    wk = nc.dram_tensor("wk", [P, FC, DPC], bf16, kind="ExternalInput")
    wv = nc.dram_tensor("wv", [P, FC, DPC], bf16, kind="ExternalInput")
    bq = nc.dram_tensor("bq", [DPC], f32, kind="ExternalInput")
    bk = nc.dram_tensor("bk", [DPC], f32, kind="ExternalInput")
    # wo packed host-side as 64 [128,512] tiles: t = (jc, hh, i)
    wo = nc.dram_tensor("wo", [P, 64, 512], bf16, kind="ExternalInput")
    ones = nc.dram_tensor("ones", [P, 512], bf16, kind="ExternalInput")
    # out rows: [0:256] = this core's batch-0 rows, [256:512] = batch-1 rows
    out = nc.dram_tensor("out", [RPC, D], bf16, kind="ExternalOutput")

    HB = RPC // B  # rows per core per batch = 256

    with tile.TileContext(nc) as tc:
        with (
            tc.tile_pool(name="persist", bufs=1) as persist,
            tc.tile_pool(name="dram", bufs=1, space="DRAM") as dram,
        ):
            # ---- persistent SBUF state ----
            qT_sb = persist.tile([P, HPC, RT], bf16)      # [hd, h, row]
            kT_sb = persist.tile([P, HPC, RT], bf16)
            v_sb = persist.tile([P, RT // P, DPC], bf16)  # [row%128, rowchunk, d]
            wo_sb = persist.tile([P, 64, 512], bf16)
            bq_sb = persist.tile([P, HPC], f32)
            bk_sb = persist.tile([P, HPC], f32)
            ones_sb = persist.tile([P, 512], bf16)

            # one A2A per (head, batch): shard j = ctx.T for batch-b rows
            # [HB*j, HB*(j+1)) in head h's feature block
            a2a_in = [[dram.tile([W, HD, HB], bf16, name=f"a2a_in{h}{b}")
                       for b in range(B)] for h in range(HPC)]
            a2a_out = [[dram.tile([W, HD, HB], bf16, name=f"a2a_out{h}{b}")
                        for b in range(B)] for h in range(HPC)]
            cs_bounce = dram.tile([HPC * B * QRC, 512], f32, name="cs_bounce")

            nc.sync.dma_start(ones_sb[:], ones.ap())
            nc.sync.dma_start(bq_sb[:], bq.ap().rearrange("(h p) -> p h", p=P))
            nc.sync.dma_start(bk_sb[:], bk.ap().rearrange("(h p) -> p h", p=P))

            # ---- HAM warmup: tiny matmuls so the PE clock gate opens
            # before the real work arrives ----
            with tc.tile_pool(name="warm_ps", bufs=1, space="PSUM") as warm_ps:
                wtile = warm_ps.tile([1, 512], f32, name="warm")
                for i in range(26):
                    nc.tensor.matmul(wtile[:], ones_sb[:, 0:1],
                                     ones_sb[:], start=True, stop=True)

            # ---- phase 1: QKV projections ----
            with (
                tc.tile_pool(name="wproj", bufs=1) as wproj,
                tc.tile_pool(name="xtp", bufs=12) as xtp,
                tc.tile_pool(name="proj_ps", bufs=1, space="PSUM") as proj_ps,
            ):
                wq_sb = wproj.tile([P, FC, DPC], bf16)
                wk_sb = wproj.tile([P, FC, DPC], bf16)
                wv_sb = wproj.tile([P, FC, DPC], bf16)
                # weight chunks on the scalar queue: first chunks arrive
                # while the x tiles stream on sync; bulk + Wo follow
                for wsb, wdr in ((wq_sb, wq), (wk_sb, wk), (wv_sb, wv)):
                    nc.scalar.dma_start(wsb[:, 0:4, :], wdr.ap()[:, 0:4, :])
                xt_pre = {}
                for fc in range(10):
                    xt = xtp.tile([P, 512], bf16, tag="xt",
                                  name=f"xt_pre{fc}")
                    xq = nc.sync if fc % 2 == 0 else nc.scalar
                    xq.dma_start(xt[:],
                                 xT.ap()[fc * P:(fc + 1) * P, 0:512])
                    xt_pre[(0, fc)] = xt
                for rc in range(RT // 512):  # 8 row chunks of 512
                    if 2 <= rc < 6:
                        # Wo staged in 0.5MB chunks spread across rc2-5 so
                        # the burst never starves the x-tile supply
                        for c2 in range(4):
                            c = (rc - 2) * 4 + c2
                            wq_ = nc.sync if c % 2 == 0 else nc.scalar
                            wq_.dma_start(
                                wo_sb[:, c * 4:(c + 1) * 4, :],
                                wo.ap()[:, c * 4:(c + 1) * 4, :])
                    q_ps = [proj_ps.tile([P, 512], f32, tag=f"q{i}",
                                         name=f"q_ps{i}")
                            for i in range(HPC)]
                    k_ps = [proj_ps.tile([P, 512], f32, tag=f"k{i}",
                                         name=f"k_ps{i}")
                            for i in range(HPC)]
                    v_ps = [proj_ps.tile([P, DPC], f32, tag=f"v{i}",
                                         name=f"v_ps{i}")
                            for i in range(4)]
                    for fc in range(FC):
                        if rc == 0 and fc in (0, 4, 8):
                            # weight chunks in groups of four, four ahead
                            for wsb, wdr in ((wq_sb, wq), (wk_sb, wk),
                                             (wv_sb, wv)):
                                nc.scalar.dma_start(
                                    wsb[:, fc + 4:fc + 8, :],
                                    wdr.ap()[:, fc + 4:fc + 8, :])
                        if (rc, fc) in xt_pre:
                            xt = xt_pre[(rc, fc)]
                        else:
                            xt = xtp.tile([P, 512], bf16, tag="xt")
                            xq = nc.sync if fc % 2 == 0 else nc.scalar
                            xq.dma_start(
                                xt[:],
                                xT.ap()[fc * P:(fc + 1) * P,
                                        rc * 512:(rc + 1) * 512])
                        st = fc == 0
                        sp = fc == FC - 1
                        # interleave short-stream V matmuls between long
                        # Q/K streams so each V LDWEIGHTS hides behind a
                        # 512-cycle stream
                        for i in range(HPC):
                            nc.tensor.matmul(
                                q_ps[i][:],
                                wq_sb[:, fc, i * HD:(i + 1) * HD],
                                xt[:], start=st, stop=sp)
                            nc.tensor.matmul(
                                v_ps[2 * i][:],
                                xt[:, 2 * i * P:(2 * i + 1) * P],
                                wv_sb[:, fc, :], start=st, stop=sp)
                            nc.tensor.matmul(
                                k_ps[i][:],
                                wk_sb[:, fc, i * HD:(i + 1) * HD],
                                xt[:], start=st, stop=sp)
                            nc.tensor.matmul(
                                v_ps[2 * i + 1][:],
                                xt[:, (2 * i + 1) * P:(2 * i + 2) * P],
                                wv_sb[:, fc, :], start=st, stop=sp)
                    # PSUM -> SBUF; Q/K on ACT (with bias), V on DVE
                    for i in range(HPC):
                        nc.scalar.activation(
                            qT_sb[:, i, rc * 512:(rc + 1) * 512],
                            q_ps[i][:], Act.Identity,
                            bias=bq_sb[:, i:i + 1])
                        nc.scalar.activation(
                            kT_sb[:, i, rc * 512:(rc + 1) * 512],
                            k_ps[i][:], Act.Identity,
                            bias=bk_sb[:, i:i + 1])
                    for s4 in range(4):
                        nc.vector.tensor_copy(
                            v_sb[:, rc * 4 + s4, :], v_ps[s4][:])

            # ctxl staging tiles (consumed by phase 3); loaded right after
            # each quarter's collective on the gpsimd queue
            ctxl_pool = tc.tile_pool(name="ctxl", bufs=1)
            ctxlp = ctxl_pool.__enter__()
            ctxl = [[ctxlp.tile([P, W, HB], bf16, name=f"ctxl{h}{b}")
                     for b in range(B)] for h in range(HPC)]

            # ---- phase 2: attention; b-outer so batch-0 re-shards early ----
            # The normalize+A2A-write stage for each qc pair is deferred by
            # one pair: the DMA-bounce broadcast round-trip then hides under
            # the next pair's 16us kc loop instead of stalling queue heads.
            with (
                tc.tile_pool(name="attn_sb", bufs=8) as attn_sb,
                tc.tile_pool(name="acc_sb", bufs=2) as acc_sbp,
                tc.tile_pool(name="norm_sb", bufs=4) as norm_sb,
                tc.tile_pool(name="st_ps", bufs=6, space="PSUM") as st_psp,
                tc.tile_pool(name="acc_ps", bufs=1, space="PSUM") as acc_psp,
            ):
                pending = None

                def flush_pending():
                    # stage B: normalize (bc is long since landed) and ship
                    # shards; fire the quarter's A2A once both pairs are in
                    nonlocal pending
                    if pending is None:
                        return
                    fh, fb, fqcs, fctxu, fbc, last = pending
                    for qc in fqcs:
                        ctxn = norm_sb.tile([P, 512], bf16, tag="ctxn")
                        nc.vector.tensor_mul(
                            ctxn[:], fctxu[qc][:], fbc[qc][:])
                        for s2 in range(2):
                            nc.gpsimd.dma_start(
                                a2a_in[fh][fb][2 * qc + s2, :, :],
                                ctxn[:, s2 * HB:(s2 + 1) * HB])
                    if last:
                        nc.gpsimd.collective_compute(
                            "AllToAll", mybir.AluOpType.bypass,
                            replica_groups=[list(range(W))],
                            ins=[a2a_in[fh][fb][:]], outs=[a2a_out[fh][fb][:]])
                    pending = None

                for b in range(B):
                    for h in range(HPC):
                        if b == 1:
                            # batch-0 ctxl loads, one head per quarter; the
                            # matching A2A completed a quarter ago, so the
                            # gpsimd queue never blocks on them
                            for i in range(W):
                                nc.gpsimd.dma_start(
                                    ctxl[h][0][:, i, :],
                                    a2a_out[h][0][i, :, :])
                        for qp in range(QRC // 2):  # qc pairs share stationaries
                            qcs = [2 * qp, 2 * qp + 1]
                            ctx_ps = {qc: acc_psp.tile(
                                [P, 512], f32,
                                tag=f"ctx{qc % 2}",
                                name=f"ctx_ps{qc % 2}")
                                      for qc in qcs}
                            # softmax denominators accumulate on DVE (pt-tile
                            # running sums), keeping the PE stream to 4
                            # matmuls per kc
                            acc_pt = {qc: acc_sbp.tile([P, 512], bf16,
                                                       tag=f"acc{qc % 2}",
                                                       name=f"acc{qc % 2}")
                                      for qc in qcs}
                            for kc in range(KRC):
                                st = kc == 0
                                sp = kc == KRC - 1
                                pts = {}
                                for qc in qcs:
                                    st_ps = st_psp.tile([P, 512], f32,
                                                        tag="st")
                                    nc.tensor.matmul(
                                        st_ps[:],
                                        kT_sb[:, h, b * SEQ + kc * P:
                                              b * SEQ + (kc + 1) * P],
                                        qT_sb[:, h, b * SEQ + qc * 512:
                                              b * SEQ + (qc + 1) * 512],
                                        start=True, stop=True)
                                    pt = attn_sb.tile([P, 512], bf16,
                                                      tag="pt")
                                    nc.scalar.activation(
                                        pt[:], st_ps[:], Act.Exp,
                                        scale=INV_SQRT_HD)
                                    pts[qc] = pt
                                for qc in qcs:
                                    nc.tensor.matmul(
                                        ctx_ps[qc][:],
                                        v_sb[:, b * KRC + kc,
                                             h * HD:(h + 1) * HD],
                                        pts[qc][:], start=st, stop=sp)
                                for qc in qcs:
                                    if kc == 0:
                                        nc.vector.tensor_copy(
                                            acc_pt[qc][:], pts[qc][:])
                                    else:
                                        nc.vector.tensor_add(
                                            acc_pt[qc][:], acc_pt[qc][:],
                                            pts[qc][:])
                            # stage A: free PSUM banks promptly and launch
                            # the reciprocal-broadcast round trip; the
                            # column-sum rides one matmul into an st slot
                            ctxu_t, bc_t = {}, {}
                            for qc in qcs:
                                cs_ps = st_psp.tile([P, 512], f32, tag="st")
                                nc.tensor.matmul(
                                    cs_ps[0:1, :], ones_sb[:, 0:1],
                                    acc_pt[qc][:], start=True, stop=True)
                                ctxu = norm_sb.tile([P, 512], f32, tag="ctxu")
                                nc.vector.tensor_copy(ctxu[:], ctx_ps[qc][:])
                                rcp = norm_sb.tile([1, 512], f32, tag="rcp")
                                nc.vector.reciprocal_approx_fast(
                                    rcp[:], cs_ps[0:1, :])
                                slot = (h * B + b) * QRC + qc
                                nc.sync.dma_start(
                                    cs_bounce[slot:slot + 1, :], rcp[:])
                                bc = norm_sb.tile([P, 512], f32, tag="bc")
                                nc.sync.dma_start(
                                    bc[:],
                                    cs_bounce[slot:slot + 1, :]
                                    .to_broadcast([P, 512]))
                                ctxu_t[qc], bc_t[qc] = ctxu, bc
                            flush_pending()
                            pending = (h, b, qcs, ctxu_t, bc_t,
                                       qp == QRC // 2 - 1)
                flush_pending()
                # batch-1 ctxl loads (gate only phase 3's second half)
                for hh in range(HPC):
                    for i in range(W):
                        nc.sync.dma_start(ctxl[hh][1][:, i, :],
                                          a2a_out[hh][1][i, :, :])

            # ---- phase 3: output projection ----
            # out rows [0:256] come from b=0 shards, [256:512] from b=1
            # stationary-outer, jc-inner: one ctxl LDWEIGHTS feeds 4 matmuls;
            # all 8 (jc, r2) accumulators live in PSUM at once
            with (
                tc.tile_pool(name="osb", bufs=4) as osbp,
                tc.tile_pool(name="o_ps", bufs=1, space="PSUM") as o_psp,
            ):
                for bb in range(B):
                    o_ps = {(jc, r2): o_psp.tile([P, 512], f32,
                                                 tag=f"o{jc}{r2}",
                                                 name=f"o_ps{jc}{r2}")
                            for jc in range(D // 512)
                            for r2 in range(HB // P)}
                    for r2 in range(HB // P):
                        for hh in range(HPC):
                            for i in range(W):
                                st = hh == 0 and i == 0
                                sp = hh == HPC - 1 and i == W - 1
                                for jc in range(D // 512):
                                    nc.tensor.matmul(
                                        o_ps[(jc, r2)][:],
                                        ctxl[hh][bb][:, i,
                                                     r2 * P:(r2 + 1) * P],
                                        wo_sb[:, jc * 16 + hh * 8 + i, :],
                                        start=st, stop=sp)
                    for jc in range(D // 512):
                        for r2 in range(HB // P):
                            o_sb = osbp.tile([P, 512], bf16, tag="osb")
                            nc.vector.tensor_copy(o_sb[:], o_ps[(jc, r2)][:])
                            nc.gpsimd.dma_start(
                                out.ap()[(bb * 2 + r2) * P:
                                         (bb * 2 + r2 + 1) * P,
                                         jc * 512:(jc + 1) * 512],
                                o_sb[:])
            ctxl_pool.__exit__(None, None, None)

    nc.compile()
    return nc


def kernel(x, Wq, bq, Wk, bk, Wv, bv, Wo, bo, _run_kwargs=None):
    global _CACHED_NC
    if _CACHED_NC is None:
        _CACHED_NC = build_nc()
    nc = _CACHED_NC

    x = np.asarray(x, dtype=np.float32)
    Wq = np.asarray(Wq, dtype=np.float32)
    Wk = np.asarray(Wk, dtype=np.float32)
    Wv = np.asarray(Wv, dtype=np.float32)
    Wo = np.asarray(Wo, dtype=np.float32)
    bq = np.asarray(bq, dtype=np.float32)
    bk = np.asarray(bk, dtype=np.float32)
    bv = np.asarray(bv, dtype=np.float32)
    bo = np.asarray(bo, dtype=np.float32)

    xT = np.ascontiguousarray(x.reshape(RT, D).T).astype(BF)   # [D, RT]
    ones = np.ones((P, 512), dtype=BF)
    bo_eff = (bo + Wo @ bv).astype(np.float32)                 # [D]

    def pack_w(Wslice):  # [DPC, D] -> [P, FC, DPC] (chunks on free axis)
        wT = Wslice.T.astype(BF)                # [D, DPC]
        return np.ascontiguousarray(
            wT.reshape(FC, P, DPC).transpose(1, 0, 2))

    # wo tiles t=(jc,hh,i): woT[i*DPC+hh*HD :+128, jc*512 :+512]
    woT = Wo.T.astype(BF)                       # [D, D]
    wo_p = np.empty((P, 64, 512), dtype=BF)
    for jc in range(4):
        for hh in range(HPC):
            for i in range(W):
                t = jc * 16 + hh * 8 + i
                r0 = i * DPC + hh * HD
                wo_p[:, t, :] = woT[r0:r0 + P, jc * 512:(jc + 1) * 512]
    wo_p = np.ascontiguousarray(wo_p)

    in_maps = []
    for i in range(W):
        sl = slice(i * DPC, (i + 1) * DPC)
        in_maps.append({
            "xT": xT,
            "wq": pack_w(Wq[sl, :]),
            "wk": pack_w(Wk[sl, :]),
            "wv": pack_w(Wv[sl, :]),
            "bq": np.ascontiguousarray(bq[sl]),
            "bk": np.ascontiguousarray(bk[sl]),
            "wo": wo_p,
            "ones": ones,
        })

    kw = _run_kwargs or {}
    res = run_bass_kernel_spmd(nc, in_maps, core_ids=list(range(W)), **kw)

    HB = RPC // B
    full = np.empty((RT, D), dtype=np.float32)
    for i in range(W):
        o = res.results[i]["out"].astype(np.float32)
        full[i * HB:(i + 1) * HB, :] = o[:HB]              # batch 0 rows
        full[SEQ + i * HB:SEQ + (i + 1) * HB, :] = o[HB:]  # batch 1 rows
    full += bo_eff[None, :]
    out = full.reshape(B, SEQ, D)
    if kw:
        kernel.last_results = res
    return out
